# revision 10
# baseline (speedup 1.0000x reference)
"""ConcatCritic pair-MLP kernel for 8 Trainium2 NeuronCores.

scores[i, j] = MLP(concat(x_i, y_j)) with
MLP = Linear(256,512) -> ReLU -> Linear(512,512) -> ReLU -> Linear(512,1).

Sharding: pure data parallelism over the B^2 pair grid, split along the
x (row) index — each core gets 64 rows of x plus all of y and the full
(small) weight set, and produces a [64, 512] slab of the output.

The host passes x and y pre-transposed and pre-cast to bf16 (xT [128,64]
slab, yT [128,512]) plus bf16 W1/W2; it receives the output in transposed
chunk layout [4][128 j][64 i]. The transposes/casts are trivial numpy work
and remove every on-device transpose/convert.

Per-core dataflow (per x-row i):
  stage A (ACT): h1T[h, j] = relu(hyT[h, j] + (hx_i[h] + b1[h])) -> bf16
  stage B (PE):  x2[j, k]  = h1_i @ W2   (bf16 matmuls, fp32 psum accum)
  stage C (DVE): s_i[j]    = sum_k relu(x2[j, k]) * W3[k]
with hxT = (x @ W1[:128]).T + b1 and hyT = (y @ W1[128:]).T computed once at
setup. b3 is applied on the host; b2 (nonzero only) via an extra K=1 matmul.

Stage B dominates: 1024 back-to-back K=128 N=512 matmuls per core. bf16
operands (vs fp32r) let FWL halve the weight load so LDWEIGHTS fully hides
under the 512-cycle stream — measured 216 ns/MM warm cadence (the exact
2.4 GHz model), where the fp32r version measured 272-280 and pushed the
package into the 2.0 GHz P0 state. Accuracy ~4e-3 absmax-rel vs the 2e-2
gate. A burst of dummy matmuls during the input-DMA ramp keeps the PE HAM
clock gate warm (4/8 -> 8/8) so the first real matmuls run at full clock;
input DMAs are spread one-tensor-per-push across four DGE rings (each
dma_start costs ~700ns of queue-engine time).
"""

import numpy as np

B = 512
DX = 128
H = 512
N_CORES = 8
ROWS = B // N_CORES  # 64 x-rows per core
HC = H // 128  # 4 chunks of the hidden dim
N_WARMUP_MM = 16  # PE pre-warm dummies (HAM un-throttle) during DMA ramp

_BUILT = {}  # with_b2 -> bass.Bass


def _build(with_b2: bool):
    import concourse.mybir as mybir
    from concourse.bacc import Bacc
    from concourse.tile import TileContext

    F32 = mybir.dt.float32
    BF16 = mybir.dt.bfloat16
    Relu = mybir.ActivationFunctionType.Relu
    Alu = mybir.AluOpType

    # Bacc (not raw Bass): its compile pipeline splits multi-semaphore waits
    # into event-semaphore chains — TRN2 engine instructions accept only one
    # sync wait, which walrus otherwise rejects.
    nc = Bacc()
    xT_d = nc.declare_dram_parameter("xT", [DX, ROWS], BF16, isOutput=False)
    yT_d = nc.declare_dram_parameter("yT", [DX, B], BF16, isOutput=False)
    w1_d = nc.declare_dram_parameter("W1bf", [2 * DX, H], BF16, isOutput=False)
    b1_d = nc.declare_dram_parameter("b1", [H], F32, isOutput=False)
    w2_d = nc.declare_dram_parameter("W2bf", [H, H], BF16, isOutput=False)
    w3_d = nc.declare_dram_parameter("W3", [H, 1], F32, isOutput=False)
    if with_b2:
        b2_d = nc.declare_dram_parameter("b2", [H], F32, isOutput=False)
    out_d = nc.declare_dram_parameter("outT", [HC, 128, ROWS], F32, isOutput=True)

    with TileContext(nc) as tc:
        with (
            tc.tile_pool(name="consts", bufs=1) as cpool,
            tc.tile_pool(name="work", bufs=2) as wpool,
            tc.tile_pool(name="psum", bufs=8, space="PSUM") as ppool,
        ):
            # ---------------- PE pre-warm ----------------
            # The HAM clock gate holds PE at 1.2 GHz until ~3.4 us of
            # sustained busy-ness, and re-throttles after a ~3.4 us idle
            # window. Dummy matmuls with no input deps bridge the input-DMA
            # ramp so the real matmuls start at full clock. Output goes to a
            # pool psum tile that is never read.
            dum_l = cpool.tile([128, 128], BF16, name="dum_l")
            dum_r = cpool.tile([128, 256], BF16, name="dum_r")
            nc.vector.memset(dum_l[:], 0.0)
            nc.vector.memset(dum_r[:], 0.0)
            ps_dum = ppool.tile([128, 256], F32, name="ps_dum", tag="ps")
            for _ in range(N_WARMUP_MM):
                nc.tensor.matmul(ps_dum[:], dum_l[:], dum_r[:], start=True, stop=True)

            # ---------------- input DMAs ----------------
            # Four DGE rings (sync/SP, scalar, vector, gpsimd) in parallel;
            # earliest-needed tensor first on each ring. Each dma_start
            # costs ~700ns on the issuing engine, so tensors are NOT split.
            # sync ring: yT (gates hy), then W2 chunks 0 and 3.
            yT = cpool.tile([DX, B], BF16, name="yT")
            nc.sync.dma_start(out=yT[:], in_=yT_d[:, :])
            w2sb = [cpool.tile([128, H], BF16, name=f"w2_{hc}") for hc in range(HC)]
            nc.sync.dma_start(out=w2sb[0][:], in_=w2_d[0:128, :])
            nc.sync.dma_start(out=w2sb[3][:], in_=w2_d[3 * 128 : 4 * 128, :])
            # scalar ring: w1y (gates hy), then W2 chunks 1 and 2.
            w1y = cpool.tile([DX, H], BF16, name="w1y")
            nc.scalar.dma_start(out=w1y[:], in_=w1_d[DX : 2 * DX, :])
            nc.scalar.dma_start(out=w2sb[1][:], in_=w2_d[128:256, :])
            nc.scalar.dma_start(out=w2sb[2][:], in_=w2_d[2 * 128 : 3 * 128, :])
            # gpsimd ring: xT and w1x (gate hx), then b1, W3 (small).
            xT = cpool.tile([DX, ROWS], BF16, name="xT")
            nc.gpsimd.dma_start(out=xT[:], in_=xT_d[:, :])
            w1x = cpool.tile([DX, H], BF16, name="w1x")
            nc.gpsimd.dma_start(out=w1x[:], in_=w1_d[0:DX, :])
            b1sb = cpool.tile([128, HC], F32, name="b1sb")  # [p, hc]
            nc.gpsimd.dma_start(out=b1sb[:], in_=b1_d[:].rearrange("(c p) -> p c", p=128))
            w3row = cpool.tile([1, H], F32, name="w3row")
            nc.gpsimd.dma_start(out=w3row[:], in_=w3_d[:, :].flatten().unsqueeze(0))
            # W3 broadcast on-chip (a partition-stride-0 DMA would re-read
            # the 2KB row 128 times from HBM).
            w3b = cpool.tile([128, H], F32, name="w3b")
            nc.gpsimd.partition_broadcast(w3b[:], w3row[:])
            if with_b2:
                b2f = cpool.tile([1, H], F32, name="b2f")
                nc.gpsimd.dma_start(out=b2f[:], in_=b2_d[:].unsqueeze(0))
                b2row = cpool.tile([1, H], BF16, name="b2row")
                nc.vector.tensor_copy(out=b2row[:], in_=b2f[:])
                ones_f = cpool.tile([1, 128], F32, name="ones_f")
                nc.vector.memset(ones_f[:], 1.0)
                ones1 = cpool.tile([1, 128], BF16, name="ones1")
                nc.vector.tensor_copy(out=ones1[:], in_=ones_f[:])

            # hxT[hc][h, i] = (x @ W1x).T + b1 — first: its DMAs (small, on
            # the gpsimd ring) land before yT/w1y, so the PE and DVE clear
            # the hx path while the y path is still streaming in.
            hxT = []
            pshx = []
            for hc in range(HC):
                ps = ppool.tile([128, ROWS], F32, name="pshx", tag="ps")
                nc.tensor.matmul(
                    ps[:], w1x[:, hc * 128 : (hc + 1) * 128], xT[:], start=True, stop=True
                )
                pshx.append(ps)
            for hc in range(HC):
                tx = cpool.tile([128, ROWS], F32, name=f"hxT_{hc}")
                nc.vector.tensor_scalar_add(tx[:], pshx[hc][:], b1sb[:, hc : hc + 1])
                hxT.append(tx)
            # hyT[hc][h, j] = (y @ W1y).T   (stored bf16 for the ACT input)
            hyT = []
            pshy = []
            for hc in range(HC):
                ps = ppool.tile([128, B], F32, name="pshy", tag="ps")
                nc.tensor.matmul(
                    ps[:], w1y[:, hc * 128 : (hc + 1) * 128], yT[:], start=True, stop=True
                )
                pshy.append(ps)
            for hc in range(HC):
                ty = cpool.tile([128, B], BF16, name=f"hyT_{hc}")
                nc.vector.tensor_copy(out=ty[:], in_=pshy[hc][:])
                hyT.append(ty)

            # scores accumulated transposed, one tile, jc-major columns:
            # scoresT[j, jc*ROWS + i] — lets a single 3D-AP DMA cover all jc.
            scoresT = cpool.tile([128, HC * ROWS], F32, name="scoresT")

            # ---------------- main loop over x rows ----------------
            for i in range(ROWS):
                h1T = []
                for hc in range(HC):
                    # ACT: relu(hyT + hx_i) -> bf16. All of stage A lives on
                    # ACT so the DVE has headroom for the stage-C reduce.
                    t = wpool.tile([128, B], BF16, name="h1T", tag="h1T", bufs=16)
                    nc.scalar.activation(
                        t[:], hyT[hc][:], Relu, bias=hxT[hc][:, i : i + 1], scale=1.0
                    )
                    h1T.append(t)
                for jc in range(HC):
                    ps2 = ppool.tile([128, B], F32, name="ps2", tag="ps")
                    for hc in range(HC):
                        nc.tensor.matmul(
                            ps2[:],
                            h1T[hc][:, jc * 128 : (jc + 1) * 128],
                            w2sb[hc][:],
                            start=(hc == 0),
                            stop=(hc == HC - 1 and not with_b2),
                        )
                    if with_b2:
                        nc.tensor.matmul(
                            ps2[:], ones1[:], b2row[:], start=False, stop=True
                        )
                    # DVE: scr = relu(ps2) * W3_bcast; scoresT col = sum_k scr
                    scr = wpool.tile([128, B], F32, name="scr", tag="scr", bufs=8)
                    nc.vector.scalar_tensor_tensor(
                        out=scr[:],
                        in0=ps2[:],
                        scalar=0.0,
                        in1=w3b[:],
                        op0=Alu.max,
                        op1=Alu.mult,
                        accum_out=scoresT[:, jc * ROWS + i : jc * ROWS + i + 1],
                    )
                if i == ROWS // 2 - 1:
                    # First half of every output chunk streams out mid-kernel
                    # (single 3D-AP push) so the tail only pays for the rest.
                    nc.sync.dma_start(
                        out=out_d.rearrange("c p i -> p c i")[:, :, 0 : ROWS // 2],
                        in_=scoresT[:].rearrange("p (c i) -> p c i", c=HC)[:, :, 0 : ROWS // 2],
                    )

            # ---------------- store (host un-transposes) ----------------
            nc.sync.dma_start(
                out=out_d.rearrange("c p i -> p c i")[:, :, ROWS // 2 :],
                in_=scoresT[:].rearrange("p (c i) -> p c i", c=HC)[:, :, ROWS // 2 :],
            )

    nc.finalize()  # runs the Bacc pass pipeline (wait splitting etc.)
    return nc


def _get_nc(with_b2: bool):
    if with_b2 not in _BUILT:
        _BUILT[with_b2] = _build(with_b2)
    return _BUILT[with_b2]


def _run(inputs: dict, trace: bool = False, **spmd_kwargs):
    """Shard, execute on 8 cores, gather. Returns (scores, BassKernelResults)."""
    import ml_dtypes
    from concourse.bass_utils import run_bass_kernel_spmd

    BF = ml_dtypes.bfloat16
    x = np.asarray(inputs["x"], dtype=np.float32)
    y = np.asarray(inputs["y"], dtype=np.float32)
    W1 = np.asarray(inputs["W1"], dtype=np.float32)
    b1 = np.ascontiguousarray(np.asarray(inputs["b1"], dtype=np.float32))
    W2 = np.asarray(inputs["W2"], dtype=np.float32)
    b2 = np.ascontiguousarray(np.asarray(inputs.get("b2", np.zeros(H)), dtype=np.float32))
    W3 = np.ascontiguousarray(np.asarray(inputs["W3"], dtype=np.float32))
    b3 = np.asarray(inputs.get("b3", np.zeros(1)), dtype=np.float32)

    with_b2 = bool(np.any(b2))
    nc = _get_nc(with_b2)

    yT = np.ascontiguousarray(y.T.astype(BF))
    W1bf = np.ascontiguousarray(W1.astype(BF))
    W2bf = np.ascontiguousarray(W2.astype(BF))
    in_maps = []
    for c in range(N_CORES):
        m = {
            "xT": np.ascontiguousarray(x[c * ROWS : (c + 1) * ROWS].T.astype(BF)),
            "yT": yT,
            "W1bf": W1bf,
            "b1": b1,
            "W2bf": W2bf,
            "W3": W3,
        }
        if with_b2:
            m["b2"] = b2
        in_maps.append(m)

    res = run_bass_kernel_spmd(
        nc, in_maps, core_ids=list(range(N_CORES)), trace=trace, **spmd_kwargs
    )
    # outT[jc, j, i] -> scores_slab[i, jc*128 + j]
    slabs = [
        np.transpose(r["outT"], (2, 0, 1)).reshape(ROWS, B) for r in res.results
    ]
    out = np.concatenate(slabs, axis=0)
    if b3.size and np.any(b3):
        out = out + b3.reshape(-1)[0]
    return np.ascontiguousarray(out.astype(np.float32)), res


def kernel(**inputs) -> np.ndarray:
    out, _ = _run(inputs)
    return out
